# revision 11
# baseline (speedup 1.0000x reference)
"""GPTQ int4 quantized linear (CaiQuantLinear) on 8 Trainium2 NeuronCores.

y = x @ dequant(qweight, scales, qzeros) + bias

Sharding: 8-way over tokens. Core c handles tokens [1024*c, +1024) and all
4096 outfeatures; the full (zero-folded) weight stream is replicated to
every core. This halves the per-core x-transpose footprint (8 MiB) vs a
4x2 token/outfeature grid, which removes the HBM front-load crunch: the
first pass only needs 8 MiB of x + 6 MiB of weights across its 55 us.

Device kernel (per core), [o,t]-oriented matmul: stationary = weight tile
[128k, 128o], moving = x tile [128k, 512t], psum [128o, 512t] (full bank).
8 passes over outfeature blocks of 512; per pass 2 token-halves x 4
o-blocks = 8 psum banks. Weights ship as host-folded (q - z) i8 plus
k-replicated bf16 scale rows; dequant is one wide DVE tensor_tensor mult
per 4-super-tile chunk. Psum is evacuated on the otherwise-idle Scalar
engine (activation Identity: out = psum + bias[o] per-partition column),
keeping the DVE free for dequant. Passes 0-1 emit b-outer (8 banks in
parallel, consuming weight/x granules as they land); passes 2-7 emit
half-serial (4 banks per token-half), spreading evacuations evenly.
"""

import sys

if "/opt/trn_rl_repo" not in sys.path:
    sys.path.insert(0, "/opt/trn_rl_repo")

import numpy as np
import ml_dtypes

import concourse.bass as bass  # noqa: F401  (registers mybir types)
import concourse.mybir as mybir
import concourse.tile as tile
from concourse import bacc
from concourse.bass_utils import run_bass_kernel_spmd

BF16 = mybir.dt.bfloat16
F32 = mybir.dt.float32
U8 = mybir.dt.uint8
I8 = mybir.dt.int8

N_CORES = 8
TOK, IN_F, OUT_F = 8192, 4096, 4096
T = TOK // N_CORES     # 1024 tokens per core
P = 128
NB = IN_F // P         # 32 contraction super-tiles
OB = 512               # outfeatures per pass (4 psum-partition blocks)
NPASS = OUT_F // OB    # 8
NH = 2                 # token halves of 512
TQ = T // NH           # 512
CB = 4                 # k-super-tiles per packed chunk
NCH = NB // CB         # 8 chunks per pass
QB = CB * OB           # 2048 q bytes per partition per chunk
CHB = QB + 2 * QB      # 6144: (q-z) i8 | s bf16

_CACHE = {}


def _build_program():
    nc = bacc.Bacc("TRN2", target_bir_lowering=False, debug=False,
                   num_devices=N_CORES)
    xt_ap = nc.dram_tensor("xt", [NCH, NH, P, CB * TQ], BF16,
                           kind="ExternalInput").ap()
    pk_ap = nc.dram_tensor("pk", [NPASS, NCH, P, CHB], U8,
                           kind="ExternalInput").ap()
    br_ap = nc.dram_tensor("br", [P, 4 * NPASS], F32, kind="ExternalInput").ap()
    y_ap = nc.dram_tensor("y", [NPASS, 4, NH, P, TQ], F32,
                          kind="ExternalOutput").ap()

    TT = mybir.AluOpType

    with tile.TileContext(nc) as tc:
        with tc.tile_pool(name="resident", bufs=1) as rpool, \
             tc.tile_pool(name="wset", bufs=2) as wpool, \
             tc.tile_pool(name="qstream", bufs=10) as qpool, \
             tc.tile_pool(name="ostream", bufs=6) as opool, \
             tc.tile_pool(name="psum", bufs=8, space="PSUM") as ppool:
            br_sb = rpool.tile([P, 4 * NPASS], F32)
            nc.gpsimd.dma_start(br_sb[:], br_ap[:])
            wz = rpool.tile([P, TQ], BF16)
            nc.vector.memset(wz[:], 0.0)
            xt_sb = rpool.tile([P, NH, NB, TQ], BF16)
            IDENT = mybir.ActivationFunctionType.Identity

            def xt_dma(ch):
                for h in range(NH):
                    eng = nc.scalar if h == 0 else nc.sync
                    eng.dma_start(xt_sb[:, h, CB * ch:CB * (ch + 1), :],
                                  xt_ap[ch, h])

            pk_tiles = {}

            def pk_dma(ob, ch):
                pk_sb = qpool.tile([P, CHB], U8, tag="pk", name=f"pk_{ob}_{ch}")
                eng = nc.scalar if ch % 2 else nc.sync
                eng.dma_start(pk_sb[:], pk_ap[ob, ch])
                pk_tiles[(ob, ch)] = pk_sb
                return pk_sb

            def deq_chunk(wset, ob, ch):
                pk_sb = pk_tiles.pop((ob, ch), None)
                if pk_sb is None:
                    pk_sb = pk_dma(ob, ch)
                nc.vector.tensor_tensor(
                    wset[:, CB * ch:CB * (ch + 1), :, :],
                    pk_sb[:, 0:QB].bitcast(I8),
                    pk_sb[:, QB:3 * QB].bitcast(BF16), TT.mult)

            def evac(pst, ob, oh, h):
                # psum + bias -> SBUF on the (otherwise idle) Scalar engine
                ot = opool.tile([P, TQ], F32, tag="ot", name=f"ot_{ob}_{oh}_{h}")
                j = 4 * ob + oh
                nc.scalar.activation(ot[:], pst[:], IDENT,
                                     bias=br_sb[:, j:j + 1])
                if ob == NPASS - 1 and h == NH - 1:
                    # final-pass tail: halve the drain latency by splitting
                    # the store across two otherwise-idle queues
                    nc.sync.dma_start(y_ap[ob, oh, h][:, :TQ // 2],
                                      ot[:, :TQ // 2])
                    nc.scalar.dma_start(y_ap[ob, oh, h][:, TQ // 2:],
                                        ot[:, TQ // 2:])
                else:
                    eng = (nc.gpsimd, nc.gpsimd, nc.sync, nc.gpsimd)[oh]
                    eng.dma_start(y_ap[ob, oh, h], ot[:])

            ps = [[None] * 4 for _ in range(NH)]

            def alloc_ps(h):
                for oh in range(4):
                    ps[h][oh] = ppool.tile([P, TQ], F32, tag="ps",
                                           name=f"ps_{h}_{oh}")

            for h in range(NH):
                alloc_ps(h)
            for oh in range(2):
                for h in range(NH):
                    nc.tensor.matmul(ps[h][oh][:], wz[:, :P], wz[:],
                                     start=True, stop=True)

            import itertools
            _wc = itertools.count()

            def wtile():
                return wpool.tile([P, NB, 4, P], BF16, tag="wset",
                                  name=f"wset_{next(_wc)}")

            def mm(wset, b, h, oh):
                nc.tensor.matmul(
                    ps[h][oh][:], wset[:, b, oh, :], xt_sb[:, h, b, :],
                    start=(b == 0), stop=(b == NB - 1))

            # ---- pass 0: b-outer, DMAs enqueued in consumption order ----
            w0 = wtile()
            for ch in range(NCH):
                if ch == 0:
                    # split the first chunk into single-super-tile quarters
                    # so the very first dequant granule lands ~4x sooner
                    pk_sb = qpool.tile([P, CHB], U8, tag="pk", name="pk_0_0")
                    HQ = QB // 4
                    for sub in range(4):
                        eng = (nc.sync, nc.scalar)[sub % 2]
                        eng.dma_start(pk_sb[:, sub * HQ:(sub + 1) * HQ],
                                      pk_ap[0, 0][:, sub * HQ:(sub + 1) * HQ])
                        eng.dma_start(
                            pk_sb[:, QB + 2 * sub * HQ:QB + 2 * (sub + 1) * HQ],
                            pk_ap[0, 0][:, QB + 2 * sub * HQ:
                                         QB + 2 * (sub + 1) * HQ])
                    xt_dma(ch)
                    for sub in range(4):
                        nc.vector.tensor_tensor(
                            w0[:, sub:sub + 1, :, :],
                            pk_sb[:, sub * HQ:(sub + 1) * HQ].bitcast(I8),
                            pk_sb[:, QB + 2 * sub * HQ:
                                  QB + 2 * (sub + 1) * HQ].bitcast(BF16),
                            TT.mult)
                else:
                    pk_dma(0, ch)
                    xt_dma(ch)
                    deq_chunk(w0, 0, ch)
                for l in range(CB):
                    b = CB * ch + l
                    for h in range(NH):
                        for oh in range(4):
                            mm(w0, b, h, oh)
                if ch >= 2:
                    pk_dma(1, ch - 2)
            pk_dma(1, 6)
            pk_dma(1, 7)
            w1 = wtile()
            deq_chunk(w1, 1, 0)
            deq_chunk(w1, 1, 1)
            for h in range(NH):
                for oh in range(4):
                    evac(ps[h][oh], 0, oh, h)
            for ch in range(2, NCH):
                deq_chunk(w1, 1, ch)

            # ---- pass 1: b-outer, wset2 production paced through it ----
            for h in range(NH):
                alloc_ps(h)
            w2 = wtile()
            for ch in range(NCH):
                for l in range(CB):
                    b = CB * ch + l
                    for h in range(NH):
                        for oh in range(4):
                            mm(w1, b, h, oh)
                if ch < 6:
                    deq_chunk(w2, 2, ch)
            deq_chunk(w2, 2, 6)
            deq_chunk(w2, 2, 7)
            for h in range(NH):
                for oh in range(4):
                    evac(ps[h][oh], 1, oh, h)

            # ---- passes 2..7: bank-serial, each bank evacuated the moment
            # it stops (spreads evacs evenly and shortens the final tail) ----
            wset_cur = w2
            for ob in range(2, NPASS):
                nxt = wtile() if ob < NPASS - 1 else None
                for h in range(NH):
                    alloc_ps(h)
                    for oh in range(4):
                        for b in range(NB):
                            mm(wset_cur, b, h, oh)
                        evac(ps[h][oh], ob, oh, h)
                        if nxt is not None:
                            deq_chunk(nxt, ob + 1, 4 * h + oh)
                wset_cur = nxt

    nc.compile()
    return nc


def _host_prep(x, qweight, scales, qzeros, bias):
    """Per-core input maps: nibble unpack + zero-fold, transpose, scale-row
    replication, dtype casts."""
    bf16 = ml_dtypes.bfloat16
    x = np.asarray(x, dtype=np.float32)
    qw = np.asarray(qweight).astype(np.int64, copy=False)
    sc = np.asarray(scales, dtype=np.float32)
    qz = np.asarray(qzeros).astype(np.int64, copy=False)
    bi = np.asarray(bias, dtype=np.float32)

    shifts = (np.arange(16, dtype=np.uint64) * np.uint64(4))
    qn = ((qw.astype(np.uint64)[:, None, :] >> shifts[None, :, None])
          & np.uint64(15)).astype(np.int16).reshape(IN_F, OUT_F)
    zz = ((qz.astype(np.uint64)[:, :, None] >> shifts[None, None, :])
          & np.uint64(15)).reshape(qz.shape[0], -1).astype(np.int16) + 1
    gidx = np.arange(IN_F) // (IN_F // qz.shape[0])
    qn = (qn - zz[gidx]).astype(np.int8)

    # weight stream, shared by all cores
    qblk = np.ascontiguousarray(
        qn.reshape(NCH, CB, P, NPASS, 4, P)
          .transpose(3, 0, 2, 1, 4, 5)
          .reshape(NPASS, NCH, P, QB)).view(np.uint8)
    s_b = sc.astype(bf16)                                # [32, OUT_F]
    a6 = s_b.reshape(NCH, CB, NPASS, 4, P).transpose(2, 0, 1, 3, 4)
    a6 = np.broadcast_to(a6[:, :, None], (NPASS, NCH, P, CB, 4, P))
    s_rep = np.ascontiguousarray(a6).view(np.uint8).reshape(
        NPASS, NCH, P, 2 * QB)
    pk = np.ascontiguousarray(np.concatenate([qblk, s_rep], axis=-1))
    br = np.ascontiguousarray(bi.reshape(4 * NPASS, P).T)

    in_maps = []
    for c in range(N_CORES):
        xs = x[c * T:(c + 1) * T]                        # [T, IN_F]
        xt = np.ascontiguousarray(xs.T).astype(bf16)     # [IN_F, T]
        xt5 = xt.reshape(NCH, CB, P, NH, TQ).transpose(0, 3, 2, 1, 4)
        in_maps.append({
            "xt": np.ascontiguousarray(xt5.reshape(NCH, NH, P, CB * TQ)),
            "pk": pk,
            "br": br,
        })
    return in_maps


def get_program():
    if "nc" not in _CACHE:
        _CACHE["nc"] = _build_program()
    return _CACHE["nc"]


def kernel(x, qweight, scales, qzeros, g_idx, bias):
    nc = get_program()
    in_maps = _host_prep(x, qweight, scales, qzeros, bias)
    res = run_bass_kernel_spmd(nc, in_maps, core_ids=list(range(N_CORES)))
    y = np.empty((TOK, OUT_F), dtype=np.float32)
    for c in range(N_CORES):
        yt = res.results[c]["y"]                 # [ob, oh, h, o, t]
        y[c * T:(c + 1) * T] = yt.transpose(2, 4, 0, 1, 3).reshape(T, OUT_F)
    return y
